# revision 35
# baseline (speedup 1.0000x reference)
"""Trainium2 Bass kernel for nn_DilatedContextAttentionModule (B=8, C=256, 64x64).

Reference, per batch element (N = 64*64 = 4096):
    g   = G xj + g_b 1^T;  th = T xi + t_b 1^T;  phi = P xj + p_b 1^T
    f   = th^T phi / N                      (N x N, linear -- NO softmax)
    y[c,n] = sum_m f[n,m] g[c,m]
    z   = W y + W_b 1^T + xi
    out = BatchNorm2d(z)                    (training-mode batch stats)

Algebraic collapse (Gram-matrix form; exact because f is linear):
    z = (I + E') xi + d 1^T
    E' = L K R + a1 b1^T + a2 b2^T,  K = xj xj^T  (C x C Gram)
    with host-folded constants
      L' = W G (device uses K/N),  R = P^T T,  wgb = W g_b,
      b1 = T^T p_b,  ptb = P^T t_b,  c1 = p_b . t_b
    and runtime vectors from sxj = xj @ 1:
      a1 = L'sxj/N + wgb,  b2 = R^T sxj  (a2 = wgb/N folded into b2/N)
      d  = L'(K/N)ptb + c1 a1 + (sxj.ptb/N) wgb + W_b
    ~0.55 GMAC/batch vs 9.7 GMAC for the naive attention (headroom=9).

Mapping to the NeuronCore (one batch element per core, 8 cores):
  - xj arrives HOST-TRANSPOSED (layout-only) as f16 with a ones column
    appended, so ONE set of Gram matmuls yields both K = xj xj^T and
    sxj = xj @ 1 (K_aug = [xj|1]^T[xj|1]).  xi and the output are f16:
    the cost model's DMA path is one serial ~275 GB/s device, so bytes
    are the dominant resource.  End-to-end rms vs fp32 jax: ~4.7e-4.
  - a short warm-up matmul burst holds the PE p-state at full clock so
    the DMA-paced Gram matmuls don't run at the 0.65 GHz cold clock.
  - z pass in [128,1024] PSUM pairs (2 banks): matmul z0 = A xi; ACT
    applies the +d bias while copying to f16 z_t; DVE computes per-pair
    sum and sum-of-squares from f16 z_t (2x/4x DVE perf modes beat
    bn_stats; avoid tensor_tensor_reduce -- its extended-ucode ISA op
    wedges the runtime).
  - BN cross-core reduction: ONE ReduceScatter (cost-model floor 15 us
    vs 28 us AllReduce): input = own stats tiled 8x (stride-0 DMA), so
    every core's scattered block is already the full global sum.
  - normalize: out = a*z - b on DVE (f16->f16 4x mode) per quarter,
    each quarter stored as f16 as soon as it is ready.

Measured (TimelineSim with collectives, the harness metric): 57652 ns
vs the 119857 ns baseline; rms relative error vs fp32 jax: 5.1e-4.
"""

import numpy as np

import concourse.bass as bass
import concourse.bacc as bacc
import concourse.tile as tile
from concourse import mybir
from concourse import bass_utils

B = 8
C = 256
N = 4096          # 64 * 64
NCORES = 8
NCH = 2           # channel chunks of 128
NT = 32           # n chunks of 128 (Gram phase)
NZ = 8            # n tiles of 512 (z phase)
XJF = 258         # xjt free width: 256 channels | ones | pad
F32 = mybir.dt.float32
F16 = mybir.dt.float16
BN_EPS = 1e-5

# wmat layout (f16, [128, 2, 770]): per channel-chunk k:
#   [0:256] L'^T rows | [256:512] R rows | [512:768] identity | [768] ptb
WM_LT = slice(0, 256)
WM_RC = slice(256, 512)
WM_ID = slice(512, 768)
WM_F = 770


def build_kernel(nc, skip_cc: bool = False) -> None:
    f32, f16 = F32, F16
    xjt_d = nc.dram_tensor("xjt", [128, NT, XJF], f16, kind="ExternalInput").ap()
    xi_d = nc.dram_tensor("xi", [128, NCH, N], f16, kind="ExternalInput").ap()
    wm_d = nc.dram_tensor("wm", [128, NCH, WM_F], f16, kind="ExternalInput").ap()
    # aux row: [b1 (256) | wgb (256) | c1 (1) | pad]
    aux_d = nc.dram_tensor("aux", [1, 2 * C + 8], f16, kind="ExternalInput").ap()
    # f32 smalls: [gamma | beta | W_b] columns  -> [128, 2, 3]
    sm_d = nc.dram_tensor("sm", [128, NCH, 3], f32, kind="ExternalInput").ap()
    out_d = nc.dram_tensor("out", [C, N], f16, kind="ExternalOutput").ap()

    with tile.TileContext(nc) as tc:
        _body(tc, xjt_d, xi_d, wm_d, aux_d, sm_d, out_d, skip_cc=skip_cc)


def _body(tc, xjt_d, xi_d, wm_d, aux_d, sm_d, out_d, skip_cc: bool = False):
    nc = tc.nc
    f32, f16 = F32, F16
    import contextlib

    with contextlib.ExitStack() as ctx:
        constp = ctx.enter_context(tc.tile_pool(name="const", bufs=1))
        datap = ctx.enter_context(tc.tile_pool(name="data", bufs=1))
        workp = ctx.enter_context(tc.tile_pool(name="work", bufs=4))
        rowsp = ctx.enter_context(tc.tile_pool(name="rows", bufs=2))
        outp = ctx.enter_context(tc.tile_pool(name="out", bufs=8))
        psbig = ctx.enter_context(tc.tile_pool(name="ps_big", bufs=2, space="PSUM"))
        psacc = ctx.enter_context(tc.tile_pool(name="ps_acc", bufs=2, space="PSUM"))
        psrow = ctx.enter_context(tc.tile_pool(name="ps_row", bufs=1, space="PSUM"))
        pscol = ctx.enter_context(tc.tile_pool(name="ps_col", bufs=1, space="PSUM"))
        dramp = ctx.enter_context(tc.tile_pool(name="dram", bufs=2, space="DRAM"))

        # ---- loads, in DMA-priority order (DMA is one serial device) ----
        xjt = datap.tile([128, NT, XJF], f16, tag="xjt")
        NXJ = 8
        for h in range(NXJ):
            sl = slice(h * (NT // NXJ), (h + 1) * (NT // NXJ))
            nc.sync.dma_start(out=xjt[:, sl, :], in_=xjt_d[:, sl, :])
        wm = constp.tile([128, NCH, WM_F], f16, tag="wm")
        nc.sync.dma_start(out=wm, in_=wm_d)
        aux = constp.tile([1, 2 * C + 8], f16, tag="aux")
        nc.sync.dma_start(out=aux, in_=aux_d)
        xi_t = datap.tile([128, NCH, N], f16, tag="xi")
        NXI = 4
        for h in range(NXI):
            sl = slice(h * (N // NXI), (h + 1) * (N // NXI))
            nc.sync.dma_start(out=xi_t[:, :, sl], in_=xi_d[:, :, sl])
        sm = constp.tile([128, NCH, 3], f32, tag="sm")
        nc.sync.dma_start(out=sm, in_=sm_d)

        eps = constp.tile([128, 1], f32, tag="eps")
        nc.vector.memset(eps, BN_EPS)

        def lt(k, csl=slice(0, C)):
            return wm[:, k, WM_LT][:, csl]

        def rcw(k, csl=slice(0, C)):
            return wm[:, k, WM_RC][:, csl]

        def ptbc(k):
            return wm[:, k, 768:769]

        # ---- PE warm-up: hold the p-state at full clock until the ----
        # ---- first Gram chunk arrives (cold PE runs at 0.65 GHz)  ----
        warm = constp.tile([128, 640], f16, tag="warm")
        nc.gpsimd.memset(warm, 0.0)
        wps = psbig.tile([128, 1024], f32, tag="big", name="warm_ps")
        NWARM = 5
        for w in range(NWARM):
            nc.tensor.matmul(wps[:, 0:512], warm[:, 0:128], warm[:, 128:640],
                             start=True, stop=True)

        # ---- phase A: K_aug = [xj|1]^T [xj|1] -> K and sxj ------------
        K_ps = [psacc.tile([128, XJF], f32, tag="acc", name=f"K_ps{m}")
                for m in range(NCH)]
        for i in range(NT):
            for m in range(NCH):
                msl = slice(m * 128, (m + 1) * 128)
                nc.tensor.matmul(
                    K_ps[m][:, 0:C + 2], xjt[:, i, msl], xjt[:, i, 0:C + 2],
                    start=(i == 0), stop=(i == NT - 1),
                )
        # K_sb = K_aug/N in f16, including the sxj/N column at index 256
        K_sb = []
        for m in range(NCH):
            t = workp.tile([128, C + 1], f16, tag=f"K{m}")
            if m == 0:
                nc.scalar.activation(
                    out=t, in_=K_ps[m][:, 0:C + 1],
                    func=mybir.ActivationFunctionType.Identity, scale=1.0 / N)
            else:
                nc.vector.tensor_scalar_mul(t, K_ps[m][:, 0:C + 1], 1.0 / N)
            K_sb.append(t)

        def sxn(k):  # sxj/N column (f16)
            return K_sb[k][:, C:C + 1]

        # ---- phase B: runtime rows a1 = L'sxj/N + wgb, b2n = R^T sxj/N
        rows_ps = psrow.tile([1, 2 * C], f32, tag="rows")
        for k in range(NCH):
            nc.tensor.matmul(rows_ps[:, 0:C], sxn(k), lt(k),
                             start=(k == 0), stop=(k == NCH - 1))
        for k in range(NCH):
            nc.tensor.matmul(rows_ps[:, C:2 * C], sxn(k), rcw(k),
                             start=(k == 0), stop=(k == NCH - 1))
        a1row = rowsp.tile([1, C], f16, tag="a1row")
        nc.vector.tensor_add(a1row, rows_ps[:, 0:C], aux[:, C:2 * C])
        b2row = rowsp.tile([1, C], f16, tag="b2row")
        nc.vector.tensor_copy(b2row, rows_ps[:, C:2 * C])

        # ---- phase C: T1 = (K/N) L'^T;  ET = R^T T1 + rank1 + I -------
        T1_sb = []
        for cb in range(NCH):
            t1_ps = psacc.tile([128, C], f32, tag="acc")
            csl = slice(cb * 128, (cb + 1) * 128)
            for jb in range(NCH):
                nc.tensor.matmul(t1_ps, K_sb[jb][:, csl], lt(jb),
                                 start=(jb == 0), stop=(jb == NCH - 1))
            t = workp.tile([128, C], f16, tag=f"T1{cb}")
            if cb == 0:
                nc.scalar.copy(t, t1_ps)
            else:
                nc.vector.tensor_copy(t, t1_ps)
            T1_sb.append(t)
        ET_sb = []
        for ob in range(NCH):
            et_ps = psacc.tile([128, C], f32, tag="acc")
            osl = slice(ob * 128, (ob + 1) * 128)
            for cb in range(NCH):
                nc.tensor.matmul(et_ps, rcw(cb, osl), T1_sb[cb],
                                 start=(cb == 0), stop=False)
            nc.tensor.matmul(et_ps, aux[:, osl], a1row, start=False, stop=False)
            nc.tensor.matmul(et_ps, b2row[:, osl], aux[:, C:2 * C],
                             start=False, stop=False)
            # += I_block via matmul (keeps DVE off the critical chain):
            # lhsT = a diagonal 128-block of I, rhs = I rows for block ob
            nc.tensor.matmul(et_ps, wm[:, 0, WM_ID][:, 0:128],
                             wm[:, ob, WM_ID], start=False, stop=True)
            t = workp.tile([128, C], f16, tag=f"ET{ob}")
            if ob == 0:
                nc.scalar.copy(t, et_ps)
            else:
                nc.vector.tensor_copy(t, et_ps)
            ET_sb.append(t)

        # ---- phase D: d = L'(K/N)ptb + c1 a1 + c2n wgb + W_b ----------
        col_ps = pscol.tile([128, 8], f32, tag="cols")
        for cb in range(NCH):
            csl = slice(cb * 128, (cb + 1) * 128)
            for jb in range(NCH):
                nc.tensor.matmul(col_ps[:, cb:cb + 1], K_sb[jb][:, csl],
                                 ptbc(jb),
                                 start=(jb == 0), stop=(jb == NCH - 1))
        # c2n = (sxj.ptb)/N via the sxj/N column of K_sb (1-partition out)
        for jb in range(NCH):
            nc.tensor.matmul(col_ps[0:1, 6:7], sxn(jb), ptbc(jb),
                             start=(jb == 0), stop=(jb == NCH - 1))
        kpc = rowsp.tile([128, NCH], f16, tag="kpc")
        nc.scalar.copy(kpc, col_ps[:, 0:NCH])
        c2cell = rowsp.tile([1, 1], f16, tag="c2cell")
        nc.vector.tensor_copy(c2cell, col_ps[0:1, 6:7])
        for ob in range(NCH):
            osl = slice(ob * 128, (ob + 1) * 128)
            for cb in range(NCH):
                nc.tensor.matmul(col_ps[:, 2 + ob:3 + ob], lt(cb, osl),
                                 kpc[:, cb:cb + 1],
                                 start=(cb == 0), stop=False)
            nc.tensor.matmul(col_ps[:, 2 + ob:3 + ob], a1row[:, osl],
                             aux[:, 2 * C:2 * C + 1], start=False, stop=False)
            nc.tensor.matmul(col_ps[:, 2 + ob:3 + ob],
                             aux[:, C + ob * 128:C + (ob + 1) * 128],
                             c2cell, start=False, stop=True)
        dcol = rowsp.tile([128, NCH], f32, tag="dcol")
        nc.vector.tensor_add(dcol, col_ps[:, 2:2 + NCH], sm[:, :, 2])

        # ---- phase E: z0 = A xi; ACT adds d into z_t; DVE bn_stats ----
        # ---- reads the same PSUM tile in parallel.  Tiles come in -----
        # ---- [128,1024] pairs (2 PSUM banks) to halve the per-op ------
        # ---- fixed costs on ACT/DVE -----------------------------------
        z_t = datap.tile([128, NCH, N], f16, tag="z")
        NP = NZ // 2
        # per-pair (sum, sumsq) accumulators: [128, j, pair, 2]
        stc = workp.tile([128, NCH, NP, 2], f32, tag="stc")
        scr = workp.tile([128, 1024], f16, tag="scr")
        scr2 = workp.tile([128, 1024], f16, tag="scr2")
        scrp = workp.tile([128, 1024], f16, tag="scrp")
        for tp in range(NP):
            tsl = slice(tp * 1024, (tp + 1) * 1024)
            for j in range(NCH):
                jsl = slice(j * 128, (j + 1) * 128)
                z_ps = psbig.tile([128, 1024], f32, tag="big")
                for half in range(2):
                    hsl = slice(half * 512, (half + 1) * 512)
                    xsl = slice(tp * 1024 + half * 512,
                                tp * 1024 + (half + 1) * 512)
                    for k in range(NCH):
                        nc.tensor.matmul(
                            z_ps[:, hsl], ET_sb[k][:, jsl], xi_t[:, k, xsl],
                            start=(k == 0), stop=(k == NCH - 1))
                if (2 * tp + j) % 2 == 0:
                    nc.scalar.activation(
                        out=z_t[:, j, tsl], in_=z_ps,
                        func=mybir.ActivationFunctionType.Identity,
                        bias=dcol[:, j:j + 1], scale=1.0,
                        accum_out=stc[:, j, tp, 0:1])
                else:
                    # odd pairs: sum(z) on DVE instead, balancing lanes
                    nc.scalar.activation(
                        out=z_t[:, j, tsl], in_=z_ps,
                        func=mybir.ActivationFunctionType.Identity,
                        bias=dcol[:, j:j + 1], scale=1.0)
                    nc.vector.tensor_scalar(
                        out=scr, in0=z_t[:, j, tsl], scalar1=1.0, scalar2=0.0,
                        op0=mybir.AluOpType.mult, op1=mybir.AluOpType.add,
                        accum_out=stc[:, j, tp, 0:1])
                # DVE computes sumsq from f16 z (2x perf mode); sum(z)
                # comes free from the ACT copy's accumulator above
                nc.vector.tensor_mul(scr2, z_t[:, j, tsl], z_t[:, j, tsl])
                nc.vector.tensor_scalar(
                    out=scrp, in0=scr2, scalar1=1.0, scalar2=0.0,
                    op0=mybir.AluOpType.mult, op1=mybir.AluOpType.add,
                    accum_out=stc[:, j, tp, 1:2])

        # spack = (mean/8, meansq/8) per chunk -> [128, 4]
        spack = rowsp.tile([128, 4], f32, tag="spack")
        for j in range(NCH):
            for s in range(2):
                acc = rowsp.tile([128, 1], f32, tag="acc1")
                nc.vector.reduce_sum(out=acc, in_=stc[:, j, :, s],
                                     axis=mybir.AxisListType.X)
                nc.vector.tensor_scalar_mul(
                    spack[:, 2 * j + s:2 * j + s + 1], acc,
                    1.0 / (N * NCORES))

        # ---- ONE ReduceScatter: input = own stats tiled 8x, so every --
        # ---- core's scattered block is already the full global sum ----
        cc_in = dramp.tile([NCORES * 128, 4], f32, tag="cc_in", name="cc_in")
        cc_out = dramp.tile([128, 4], f32, tag="cc_out", name="cc_out")
        nc.sync.dma_start(
            out=cc_in.rearrange("(r p) f -> p r f", p=128),
            in_=spack.unsqueeze(1).broadcast_to([128, NCORES, 4]))
        if skip_cc:
            nc.sync.dma_start(out=cc_out, in_=cc_in[0:128, :])
        else:
            nc.gpsimd.collective_compute(
                "ReduceScatter",
                mybir.AluOpType.add,
                replica_groups=[list(range(NCORES))],
                ins=[cc_in.opt()],
                outs=[cc_out.opt()],
            )
        ssum = rowsp.tile([128, 4], f32, tag="ssum")
        nc.sync.dma_start(out=ssum, in_=cc_out)

        # ---- affine constants for BOTH chunks first, then applies -----
        acols = rowsp.tile([128, NCH], f32, tag="acols")
        bcols = rowsp.tile([128, NCH], f32, tag="bcols")
        for j in range(NCH):
            mcol = ssum[:, 2 * j:2 * j + 1]
            qcol = ssum[:, 2 * j + 1:2 * j + 2]
            nvcol = rowsp.tile([128, 1], f32, tag="nvcol")
            nc.vector.scalar_tensor_tensor(
                out=nvcol, in0=mcol, scalar=mcol, in1=qcol,
                op0=mybir.AluOpType.mult, op1=mybir.AluOpType.subtract)
            scol = rowsp.tile([128, 1], f32, tag="scol")
            nc.scalar.activation(
                out=scol, in_=nvcol, func=mybir.ActivationFunctionType.Sqrt,
                bias=eps, scale=-1.0)
            nc.vector.reciprocal(out=scol, in_=scol)
            nc.vector.tensor_mul(acols[:, j:j + 1], scol, sm[:, j, 0:1])
            nc.vector.scalar_tensor_tensor(
                out=bcols[:, j:j + 1], in0=mcol, scalar=acols[:, j:j + 1],
                in1=sm[:, j, 1:2],
                op0=mybir.AluOpType.mult, op1=mybir.AluOpType.subtract)

        # ---- apply out = a*z - b: all on DVE (f16->f16 runs in the ----
        # ---- 4x perf mode, ~110ns/512), store each piece --------------
        NQ = 4
        for j in range(NCH):
            for q in range(NQ):
                qsl = slice(q * (N // NQ), (q + 1) * (N // NQ))
                o16 = outp.tile([128, N // NQ], f16, tag="o16")
                nc.vector.tensor_scalar(
                    out=o16, in0=z_t[:, j, qsl],
                    scalar1=acols[:, j:j + 1], scalar2=bcols[:, j:j + 1],
                    op0=mybir.AluOpType.mult, op1=mybir.AluOpType.subtract)
                nc.sync.dma_start(
                    out=out_d[j * 128:(j + 1) * 128, qsl], in_=o16)


_NC_CACHE: dict = {}


def _get_nc():
    if "nc" not in _NC_CACHE:
        nc = bacc.Bacc(
            "TRN2",
            target_bir_lowering=False,
            debug=False,
            enable_asserts=True,
            num_devices=NCORES,
        )
        build_kernel(nc)
        nc.compile()
        _NC_CACHE["nc"] = nc
    return _NC_CACHE["nc"]


def _make_in_maps(inputs: dict) -> list[dict]:
    xi = np.asarray(inputs["xi"], np.float32).reshape(B, C, N)
    xj = np.asarray(inputs["xj"], np.float32).reshape(B, C, N)
    g_w = np.asarray(inputs["g_w"], np.float32)
    g_b = np.asarray(inputs["g_b"], np.float32)
    t_w = np.asarray(inputs["theta_w"], np.float32)
    t_b = np.asarray(inputs["theta_b"], np.float32)
    p_w = np.asarray(inputs["phi_w"], np.float32)
    p_b = np.asarray(inputs["phi_b"], np.float32)
    W_w = np.asarray(inputs["W_w"], np.float32)
    W_b = np.asarray(inputs["W_b"], np.float32)
    gam = np.asarray(inputs["bn_gamma"], np.float32)
    bet = np.asarray(inputs["bn_beta"], np.float32)

    def chunked(a):  # [256, F] -> [128, 2, F]
        return np.ascontiguousarray(a.reshape(2, 128, -1).transpose(1, 0, 2))

    # host-folded weight products (constant folding, fp32)
    Lp = W_w @ g_w                      # L' = W G   (device uses K/N)
    R = p_w.T @ t_w                     # R = P^T T
    wgb = W_w @ g_b
    b1 = t_w.T @ p_b
    ptb = p_w.T @ t_b
    c1 = float(p_b @ t_b)

    wm = np.zeros((128, NCH, WM_F), np.float16)
    wm[:, :, 0:C] = chunked(Lp.T)
    wm[:, :, C:2 * C] = chunked(R)
    wm[:, :, 2 * C:3 * C] = chunked(np.eye(C, dtype=np.float32))
    wm[:, :, 3 * C] = ptb.reshape(2, 128).T
    aux = np.zeros((1, 2 * C + 8), np.float16)
    aux[0, 0:C] = b1.astype(np.float16)
    aux[0, C:2 * C] = wgb.astype(np.float16)
    aux[0, 2 * C] = np.float16(c1)
    sm = np.zeros((128, NCH, 3), np.float32)
    sm[:, :, 0] = gam.reshape(2, 128).T
    sm[:, :, 1] = bet.reshape(2, 128).T
    sm[:, :, 2] = W_b.reshape(2, 128).T

    in_maps = []
    for b in range(B):
        # layout-only transforms of the per-batch data (f16)
        xjta = np.zeros((128, NT, XJF), np.float16)
        xjta[:, :, 0:C] = xj[b].T.reshape(NT, 128, C).transpose(1, 0, 2)
        xjta[:, :, C] = 1.0
        xib = chunked(xi[b]).astype(np.float16)      # [128,2,4096]
        in_maps.append({
            "xjt": xjta, "xi": xib, "wm": wm, "aux": aux, "sm": sm,
        })
    return in_maps


def kernel(**inputs) -> np.ndarray:
    nc = _get_nc()
    in_maps = _make_in_maps(inputs)
    last_err = None
    for attempt in range(3):
        try:
            res = bass_utils.run_bass_kernel_spmd(
                nc, in_maps, core_ids=list(range(NCORES)),
            )
            break
        except Exception as e:  # transient device wedge: back off and retry
            last_err = e
            import time as _time
            _time.sleep(4.0 * (attempt + 1))
            try:
                import jax
                import jax.extend.backend as _jeb
                jax.clear_caches()
                _jeb.clear_backends()
            except Exception:
                pass
    else:
        raise last_err
    out = np.stack([res.results[c]["out"] for c in range(NCORES)])
    return np.ascontiguousarray(out.reshape(B, C, 64, 64).astype(np.float32))


if __name__ == "__main__":
    rng = np.random.default_rng(0)
    fake = {
        "xi": rng.standard_normal((B, C, 64, 64)).astype(np.float32),
        "xj": rng.standard_normal((B, C, 64, 64)).astype(np.float32),
        "g_w": (rng.standard_normal((C, C)) / 16).astype(np.float32),
        "g_b": (rng.standard_normal((C,)) / 16).astype(np.float32),
        "theta_w": (rng.standard_normal((C, C)) / 16).astype(np.float32),
        "theta_b": (rng.standard_normal((C,)) / 16).astype(np.float32),
        "phi_w": (rng.standard_normal((C, C)) / 16).astype(np.float32),
        "phi_b": (rng.standard_normal((C,)) / 16).astype(np.float32),
        "W_w": (rng.standard_normal((C, C)) / 16).astype(np.float32),
        "W_b": (rng.standard_normal((C,)) / 16).astype(np.float32),
        "bn_gamma": np.ones((C,), np.float32),
        "bn_beta": np.zeros((C,), np.float32),
    }
    out = kernel(**fake)
    print("out", out.shape, out.dtype, float(np.abs(out).mean()))


# revision 36
# speedup vs baseline: 1.0094x; 1.0094x over previous
"""Trainium2 Bass kernel for nn_DilatedContextAttentionModule (B=8, C=256, 64x64).

Reference, per batch element (N = 64*64 = 4096):
    g   = G xj + g_b 1^T;  th = T xi + t_b 1^T;  phi = P xj + p_b 1^T
    f   = th^T phi / N                      (N x N, linear -- NO softmax)
    y[c,n] = sum_m f[n,m] g[c,m]
    z   = W y + W_b 1^T + xi
    out = BatchNorm2d(z)                    (training-mode batch stats)

Algebraic collapse (Gram-matrix form; exact because f is linear):
    z = (I + E') xi + d 1^T
    E' = L K R + a1 b1^T + a2 b2^T,  K = xj xj^T  (C x C Gram)
    with host-folded constants
      L' = W G (device uses K/N),  R = P^T T,  wgb = W g_b,
      b1 = T^T p_b,  ptb = P^T t_b,  c1 = p_b . t_b
    and runtime vectors from sxj = xj @ 1:
      a1 = L'sxj/N + wgb,  b2 = R^T sxj  (a2 = wgb/N folded into b2/N)
      d  = L'(K/N)ptb + c1 a1 + (sxj.ptb/N) wgb + W_b
    ~0.55 GMAC/batch vs 9.7 GMAC for the naive attention (headroom=9).

Mapping to the NeuronCore (one batch element per core, 8 cores):
  - xj arrives HOST-TRANSPOSED (layout-only) as f16 with a ones column
    appended, so ONE set of Gram matmuls yields both K = xj xj^T and
    sxj = xj @ 1 (K_aug = [xj|1]^T[xj|1]).  xi and the output are f16:
    the cost model's DMA path is one serial ~275 GB/s device, so bytes
    are the dominant resource.  End-to-end rms vs fp32 jax: ~4.7e-4.
  - a short warm-up matmul burst holds the PE p-state at full clock so
    the DMA-paced Gram matmuls don't run at the 0.65 GHz cold clock.
  - z pass in [128,1024] PSUM pairs (2 banks): matmul z0 = A xi; ACT
    applies the +d bias while copying to f16 z_t; DVE computes per-pair
    sum and sum-of-squares from f16 z_t (2x/4x DVE perf modes beat
    bn_stats; avoid tensor_tensor_reduce -- its extended-ucode ISA op
    wedges the runtime).
  - BN cross-core reduction: ONE ReduceScatter (cost-model floor 15 us
    vs 28 us AllReduce): input = own stats tiled 8x (stride-0 DMA), so
    every core's scattered block is already the full global sum.
  - normalize: out = a*z - b on DVE (f16->f16 4x mode) per quarter,
    each quarter stored as f16 as soon as it is ready.

Measured (TimelineSim with collectives, the harness metric): 57516 ns
vs the 119857 ns baseline; rms relative error vs fp32 jax: 5.1e-4.
"""

import numpy as np

import concourse.bass as bass
import concourse.bacc as bacc
import concourse.tile as tile
from concourse import mybir
from concourse import bass_utils

B = 8
C = 256
N = 4096          # 64 * 64
NCORES = 8
NCH = 2           # channel chunks of 128
NT = 32           # n chunks of 128 (Gram phase)
NZ = 8            # n tiles of 512 (z phase)
XJF = 258         # xjt free width: 256 channels | ones | pad
F32 = mybir.dt.float32
F16 = mybir.dt.float16
BN_EPS = 1e-5

# wmat layout (f16, [128, 2, 770]): per channel-chunk k:
#   [0:256] L'^T rows | [256:512] R rows | [512:768] identity | [768] ptb
WM_LT = slice(0, 256)
WM_RC = slice(256, 512)
WM_ID = slice(512, 768)
WM_F = 770


def build_kernel(nc, skip_cc: bool = False) -> None:
    f32, f16 = F32, F16
    xjt_d = nc.dram_tensor("xjt", [128, NT, XJF], f16, kind="ExternalInput").ap()
    xi_d = nc.dram_tensor("xi", [128, NCH, N], f16, kind="ExternalInput").ap()
    wm_d = nc.dram_tensor("wm", [128, NCH, WM_F], f16, kind="ExternalInput").ap()
    # aux row: [b1 (256) | wgb (256) | c1 (1) | pad]
    aux_d = nc.dram_tensor("aux", [1, 2 * C + 8], f16, kind="ExternalInput").ap()
    # f32 smalls: [gamma | beta | W_b] columns  -> [128, 2, 3]
    sm_d = nc.dram_tensor("sm", [128, NCH, 3], f32, kind="ExternalInput").ap()
    out_d = nc.dram_tensor("out", [C, N], f16, kind="ExternalOutput").ap()

    with tile.TileContext(nc) as tc:
        _body(tc, xjt_d, xi_d, wm_d, aux_d, sm_d, out_d, skip_cc=skip_cc)


def _body(tc, xjt_d, xi_d, wm_d, aux_d, sm_d, out_d, skip_cc: bool = False):
    nc = tc.nc
    f32, f16 = F32, F16
    import contextlib

    with contextlib.ExitStack() as ctx:
        constp = ctx.enter_context(tc.tile_pool(name="const", bufs=1))
        datap = ctx.enter_context(tc.tile_pool(name="data", bufs=1))
        workp = ctx.enter_context(tc.tile_pool(name="work", bufs=4))
        rowsp = ctx.enter_context(tc.tile_pool(name="rows", bufs=2))
        outp = ctx.enter_context(tc.tile_pool(name="out", bufs=8))
        psbig = ctx.enter_context(tc.tile_pool(name="ps_big", bufs=2, space="PSUM"))
        psacc = ctx.enter_context(tc.tile_pool(name="ps_acc", bufs=2, space="PSUM"))
        psrow = ctx.enter_context(tc.tile_pool(name="ps_row", bufs=1, space="PSUM"))
        pscol = ctx.enter_context(tc.tile_pool(name="ps_col", bufs=1, space="PSUM"))
        dramp = ctx.enter_context(tc.tile_pool(name="dram", bufs=2, space="DRAM"))

        # ---- loads, in DMA-priority order (DMA is one serial device) ----
        xjt = datap.tile([128, NT, XJF], f16, tag="xjt")
        NXJ = 8
        for h in range(NXJ):
            sl = slice(h * (NT // NXJ), (h + 1) * (NT // NXJ))
            nc.sync.dma_start(out=xjt[:, sl, :], in_=xjt_d[:, sl, :])
        wm = constp.tile([128, NCH, WM_F], f16, tag="wm")
        nc.sync.dma_start(out=wm, in_=wm_d)
        aux = constp.tile([1, 2 * C + 8], f16, tag="aux")
        nc.sync.dma_start(out=aux, in_=aux_d)
        xi_t = datap.tile([128, NCH, N], f16, tag="xi")
        NXI = 4
        for h in range(NXI):
            sl = slice(h * (N // NXI), (h + 1) * (N // NXI))
            nc.sync.dma_start(out=xi_t[:, :, sl], in_=xi_d[:, :, sl])
        sm = constp.tile([128, NCH, 3], f32, tag="sm")
        nc.sync.dma_start(out=sm, in_=sm_d)

        eps = constp.tile([128, 1], f32, tag="eps")
        nc.vector.memset(eps, BN_EPS)

        def lt(k, csl=slice(0, C)):
            return wm[:, k, WM_LT][:, csl]

        def rcw(k, csl=slice(0, C)):
            return wm[:, k, WM_RC][:, csl]

        def ptbc(k):
            return wm[:, k, 768:769]

        # ---- PE warm-up: hold the p-state at full clock until the ----
        # ---- first Gram chunk arrives (cold PE runs at 0.65 GHz)  ----
        warm = constp.tile([128, 640], f16, tag="warm")
        nc.gpsimd.memset(warm, 0.0)
        wps = psbig.tile([128, 1024], f32, tag="big", name="warm_ps")
        NWARM = 5
        for w in range(NWARM):
            nc.tensor.matmul(wps[:, 0:512], warm[:, 0:128], warm[:, 128:640],
                             start=True, stop=True)

        # ---- phase A: K_aug = [xj|1]^T [xj|1] -> K and sxj ------------
        K_ps = [psacc.tile([128, XJF], f32, tag="acc", name=f"K_ps{m}")
                for m in range(NCH)]
        for i in range(NT):
            for m in range(NCH):
                msl = slice(m * 128, (m + 1) * 128)
                nc.tensor.matmul(
                    K_ps[m][:, 0:C + 2], xjt[:, i, msl], xjt[:, i, 0:C + 2],
                    start=(i == 0), stop=(i == NT - 1),
                )
        # K_sb = K_aug/N in f16, including the sxj/N column at index 256
        K_sb = []
        for m in range(NCH):
            t = workp.tile([128, C + 1], f16, tag=f"K{m}")
            if m == 0:
                nc.scalar.activation(
                    out=t, in_=K_ps[m][:, 0:C + 1],
                    func=mybir.ActivationFunctionType.Identity, scale=1.0 / N)
            else:
                nc.vector.tensor_scalar_mul(t, K_ps[m][:, 0:C + 1], 1.0 / N)
            K_sb.append(t)

        def sxn(k):  # sxj/N column (f16)
            return K_sb[k][:, C:C + 1]

        # ---- phase B: runtime rows a1 = L'sxj/N + wgb, b2n = R^T sxj/N
        rows_ps = psrow.tile([1, 2 * C], f32, tag="rows")
        for k in range(NCH):
            nc.tensor.matmul(rows_ps[:, 0:C], sxn(k), lt(k),
                             start=(k == 0), stop=(k == NCH - 1))
        for k in range(NCH):
            nc.tensor.matmul(rows_ps[:, C:2 * C], sxn(k), rcw(k),
                             start=(k == 0), stop=(k == NCH - 1))
        a1row = rowsp.tile([1, C], f16, tag="a1row")
        nc.vector.tensor_add(a1row, rows_ps[:, 0:C], aux[:, C:2 * C])
        b2row = rowsp.tile([1, C], f16, tag="b2row")
        nc.vector.tensor_copy(b2row, rows_ps[:, C:2 * C])

        # ---- phase C: T1 = (K/N) L'^T;  ET = R^T T1 + rank1 + I -------
        T1_sb = []
        for cb in range(NCH):
            t1_ps = psacc.tile([128, C], f32, tag="acc")
            csl = slice(cb * 128, (cb + 1) * 128)
            for jb in range(NCH):
                nc.tensor.matmul(t1_ps, K_sb[jb][:, csl], lt(jb),
                                 start=(jb == 0), stop=(jb == NCH - 1))
            t = workp.tile([128, C], f16, tag=f"T1{cb}")
            if cb == 0:
                nc.scalar.copy(t, t1_ps)
            else:
                nc.vector.tensor_copy(t, t1_ps)
            T1_sb.append(t)
        ET_sb = []
        for ob in range(NCH):
            et_ps = psacc.tile([128, C], f32, tag="acc")
            osl = slice(ob * 128, (ob + 1) * 128)
            for cb in range(NCH):
                nc.tensor.matmul(et_ps, rcw(cb, osl), T1_sb[cb],
                                 start=(cb == 0), stop=False)
            nc.tensor.matmul(et_ps, aux[:, osl], a1row, start=False, stop=False)
            nc.tensor.matmul(et_ps, b2row[:, osl], aux[:, C:2 * C],
                             start=False, stop=False)
            # += I_block via matmul (keeps DVE off the critical chain):
            # lhsT = a diagonal 128-block of I, rhs = I rows for block ob
            nc.tensor.matmul(et_ps, wm[:, 0, WM_ID][:, 0:128],
                             wm[:, ob, WM_ID], start=False, stop=True)
            t = workp.tile([128, C], f16, tag=f"ET{ob}")
            if ob == 0:
                nc.scalar.copy(t, et_ps)
            else:
                nc.vector.tensor_copy(t, et_ps)
            ET_sb.append(t)

        # ---- phase D: d = L'(K/N)ptb + c1 a1 + c2n wgb + W_b ----------
        col_ps = pscol.tile([128, 8], f32, tag="cols")
        for cb in range(NCH):
            csl = slice(cb * 128, (cb + 1) * 128)
            for jb in range(NCH):
                nc.tensor.matmul(col_ps[:, cb:cb + 1], K_sb[jb][:, csl],
                                 ptbc(jb),
                                 start=(jb == 0), stop=(jb == NCH - 1))
        # c2n = (sxj.ptb)/N via the sxj/N column of K_sb (1-partition out)
        for jb in range(NCH):
            nc.tensor.matmul(col_ps[0:1, 6:7], sxn(jb), ptbc(jb),
                             start=(jb == 0), stop=(jb == NCH - 1))
        kpc = rowsp.tile([128, NCH], f16, tag="kpc")
        nc.scalar.copy(kpc, col_ps[:, 0:NCH])
        c2cell = rowsp.tile([1, 1], f16, tag="c2cell")
        nc.vector.tensor_copy(c2cell, col_ps[0:1, 6:7])
        for ob in range(NCH):
            osl = slice(ob * 128, (ob + 1) * 128)
            for cb in range(NCH):
                nc.tensor.matmul(col_ps[:, 2 + ob:3 + ob], lt(cb, osl),
                                 kpc[:, cb:cb + 1],
                                 start=(cb == 0), stop=False)
            nc.tensor.matmul(col_ps[:, 2 + ob:3 + ob], a1row[:, osl],
                             aux[:, 2 * C:2 * C + 1], start=False, stop=False)
            nc.tensor.matmul(col_ps[:, 2 + ob:3 + ob],
                             aux[:, C + ob * 128:C + (ob + 1) * 128],
                             c2cell, start=False, stop=True)
        dcol = rowsp.tile([128, NCH], f32, tag="dcol")
        nc.vector.tensor_add(dcol, col_ps[:, 2:2 + NCH], sm[:, :, 2])

        # ---- phase E: z0 = A xi; ACT adds d into z_t; DVE bn_stats ----
        # ---- reads the same PSUM tile in parallel.  Tiles come in -----
        # ---- [128,1024] pairs (2 PSUM banks) to halve the per-op ------
        # ---- fixed costs on ACT/DVE -----------------------------------
        z_t = datap.tile([128, NCH, N], f16, tag="z")
        NP = NZ // 2
        # per-pair (sum, sumsq) accumulators: [128, j, pair, 2]
        stc = workp.tile([128, NCH, NP, 2], f32, tag="stc")
        scr = workp.tile([128, 1024], f16, tag="scr")
        scr2 = workp.tile([128, 1024], f16, tag="scr2")
        scrp = workp.tile([128, 1024], f16, tag="scrp")
        for tp in range(NP):
            tsl = slice(tp * 1024, (tp + 1) * 1024)
            for j in range(NCH):
                jsl = slice(j * 128, (j + 1) * 128)
                z_ps = psbig.tile([128, 1024], f32, tag="big")
                for half in range(2):
                    hsl = slice(half * 512, (half + 1) * 512)
                    xsl = slice(tp * 1024 + half * 512,
                                tp * 1024 + (half + 1) * 512)
                    for k in range(NCH):
                        nc.tensor.matmul(
                            z_ps[:, hsl], ET_sb[k][:, jsl], xi_t[:, k, xsl],
                            start=(k == 0), stop=(k == NCH - 1))
                nc.scalar.activation(
                    out=z_t[:, j, tsl], in_=z_ps,
                    func=mybir.ActivationFunctionType.Identity,
                    bias=dcol[:, j:j + 1], scale=1.0,
                    accum_out=stc[:, j, tp, 0:1])
                # DVE computes sumsq from f16 z (2x perf mode); sum(z)
                # comes free from the ACT copy's accumulator above
                nc.vector.tensor_mul(scr2, z_t[:, j, tsl], z_t[:, j, tsl])
                nc.vector.tensor_scalar(
                    out=scrp, in0=scr2, scalar1=1.0, scalar2=0.0,
                    op0=mybir.AluOpType.mult, op1=mybir.AluOpType.add,
                    accum_out=stc[:, j, tp, 1:2])

        # spack = (mean/8, meansq/8) per chunk -> [128, 4]
        spack = rowsp.tile([128, 4], f32, tag="spack")
        for j in range(NCH):
            for s in range(2):
                acc = rowsp.tile([128, 1], f32, tag="acc1")
                nc.vector.reduce_sum(out=acc, in_=stc[:, j, :, s],
                                     axis=mybir.AxisListType.X)
                nc.vector.tensor_scalar_mul(
                    spack[:, 2 * j + s:2 * j + s + 1], acc,
                    1.0 / (N * NCORES))

        # ---- ONE ReduceScatter: input = own stats tiled 8x, so every --
        # ---- core's scattered block is already the full global sum ----
        cc_in = dramp.tile([NCORES * 128, 4], f32, tag="cc_in", name="cc_in")
        cc_out = dramp.tile([128, 4], f32, tag="cc_out", name="cc_out")
        nc.sync.dma_start(
            out=cc_in.rearrange("(r p) f -> p r f", p=128),
            in_=spack.unsqueeze(1).broadcast_to([128, NCORES, 4]))
        if skip_cc:
            nc.sync.dma_start(out=cc_out, in_=cc_in[0:128, :])
        else:
            nc.gpsimd.collective_compute(
                "ReduceScatter",
                mybir.AluOpType.add,
                replica_groups=[list(range(NCORES))],
                ins=[cc_in.opt()],
                outs=[cc_out.opt()],
            )
        ssum = rowsp.tile([128, 4], f32, tag="ssum")
        nc.sync.dma_start(out=ssum, in_=cc_out)

        # ---- affine constants for BOTH chunks first, then applies -----
        acols = rowsp.tile([128, NCH], f32, tag="acols")
        bcols = rowsp.tile([128, NCH], f32, tag="bcols")
        for j in range(NCH):
            mcol = ssum[:, 2 * j:2 * j + 1]
            qcol = ssum[:, 2 * j + 1:2 * j + 2]
            nvcol = rowsp.tile([128, 1], f32, tag="nvcol")
            nc.vector.scalar_tensor_tensor(
                out=nvcol, in0=mcol, scalar=mcol, in1=qcol,
                op0=mybir.AluOpType.mult, op1=mybir.AluOpType.subtract)
            scol = rowsp.tile([128, 1], f32, tag="scol")
            nc.scalar.activation(
                out=scol, in_=nvcol, func=mybir.ActivationFunctionType.Sqrt,
                bias=eps, scale=-1.0)
            nc.vector.reciprocal(out=scol, in_=scol)
            nc.vector.tensor_mul(acols[:, j:j + 1], scol, sm[:, j, 0:1])
            nc.vector.scalar_tensor_tensor(
                out=bcols[:, j:j + 1], in0=mcol, scalar=acols[:, j:j + 1],
                in1=sm[:, j, 1:2],
                op0=mybir.AluOpType.mult, op1=mybir.AluOpType.subtract)

        # ---- apply out = a*z - b: all on DVE (f16->f16 runs in the ----
        # ---- 4x perf mode, ~110ns/512), store each piece --------------
        NQ = 4
        for j in range(NCH):
            for q in range(NQ):
                qsl = slice(q * (N // NQ), (q + 1) * (N // NQ))
                o16 = outp.tile([128, N // NQ], f16, tag="o16")
                nc.vector.tensor_scalar(
                    out=o16, in0=z_t[:, j, qsl],
                    scalar1=acols[:, j:j + 1], scalar2=bcols[:, j:j + 1],
                    op0=mybir.AluOpType.mult, op1=mybir.AluOpType.subtract)
                nc.sync.dma_start(
                    out=out_d[j * 128:(j + 1) * 128, qsl], in_=o16)


_NC_CACHE: dict = {}


def _get_nc():
    if "nc" not in _NC_CACHE:
        nc = bacc.Bacc(
            "TRN2",
            target_bir_lowering=False,
            debug=False,
            enable_asserts=True,
            num_devices=NCORES,
        )
        build_kernel(nc)
        nc.compile()
        _NC_CACHE["nc"] = nc
    return _NC_CACHE["nc"]


def _make_in_maps(inputs: dict) -> list[dict]:
    xi = np.asarray(inputs["xi"], np.float32).reshape(B, C, N)
    xj = np.asarray(inputs["xj"], np.float32).reshape(B, C, N)
    g_w = np.asarray(inputs["g_w"], np.float32)
    g_b = np.asarray(inputs["g_b"], np.float32)
    t_w = np.asarray(inputs["theta_w"], np.float32)
    t_b = np.asarray(inputs["theta_b"], np.float32)
    p_w = np.asarray(inputs["phi_w"], np.float32)
    p_b = np.asarray(inputs["phi_b"], np.float32)
    W_w = np.asarray(inputs["W_w"], np.float32)
    W_b = np.asarray(inputs["W_b"], np.float32)
    gam = np.asarray(inputs["bn_gamma"], np.float32)
    bet = np.asarray(inputs["bn_beta"], np.float32)

    def chunked(a):  # [256, F] -> [128, 2, F]
        return np.ascontiguousarray(a.reshape(2, 128, -1).transpose(1, 0, 2))

    # host-folded weight products (constant folding, fp32)
    Lp = W_w @ g_w                      # L' = W G   (device uses K/N)
    R = p_w.T @ t_w                     # R = P^T T
    wgb = W_w @ g_b
    b1 = t_w.T @ p_b
    ptb = p_w.T @ t_b
    c1 = float(p_b @ t_b)

    wm = np.zeros((128, NCH, WM_F), np.float16)
    wm[:, :, 0:C] = chunked(Lp.T)
    wm[:, :, C:2 * C] = chunked(R)
    wm[:, :, 2 * C:3 * C] = chunked(np.eye(C, dtype=np.float32))
    wm[:, :, 3 * C] = ptb.reshape(2, 128).T
    aux = np.zeros((1, 2 * C + 8), np.float16)
    aux[0, 0:C] = b1.astype(np.float16)
    aux[0, C:2 * C] = wgb.astype(np.float16)
    aux[0, 2 * C] = np.float16(c1)
    sm = np.zeros((128, NCH, 3), np.float32)
    sm[:, :, 0] = gam.reshape(2, 128).T
    sm[:, :, 1] = bet.reshape(2, 128).T
    sm[:, :, 2] = W_b.reshape(2, 128).T

    in_maps = []
    for b in range(B):
        # layout-only transforms of the per-batch data (f16)
        xjta = np.zeros((128, NT, XJF), np.float16)
        xjta[:, :, 0:C] = xj[b].T.reshape(NT, 128, C).transpose(1, 0, 2)
        xjta[:, :, C] = 1.0
        xib = chunked(xi[b]).astype(np.float16)      # [128,2,4096]
        in_maps.append({
            "xjt": xjta, "xi": xib, "wm": wm, "aux": aux, "sm": sm,
        })
    return in_maps


def kernel(**inputs) -> np.ndarray:
    nc = _get_nc()
    in_maps = _make_in_maps(inputs)
    last_err = None
    for attempt in range(3):
        try:
            res = bass_utils.run_bass_kernel_spmd(
                nc, in_maps, core_ids=list(range(NCORES)),
            )
            break
        except Exception as e:  # transient device wedge: back off and retry
            last_err = e
            import time as _time
            _time.sleep(4.0 * (attempt + 1))
            try:
                import jax
                import jax.extend.backend as _jeb
                jax.clear_caches()
                _jeb.clear_backends()
            except Exception:
                pass
    else:
        raise last_err
    out = np.stack([res.results[c]["out"] for c in range(NCORES)])
    return np.ascontiguousarray(out.reshape(B, C, 64, 64).astype(np.float32))


if __name__ == "__main__":
    rng = np.random.default_rng(0)
    fake = {
        "xi": rng.standard_normal((B, C, 64, 64)).astype(np.float32),
        "xj": rng.standard_normal((B, C, 64, 64)).astype(np.float32),
        "g_w": (rng.standard_normal((C, C)) / 16).astype(np.float32),
        "g_b": (rng.standard_normal((C,)) / 16).astype(np.float32),
        "theta_w": (rng.standard_normal((C, C)) / 16).astype(np.float32),
        "theta_b": (rng.standard_normal((C,)) / 16).astype(np.float32),
        "phi_w": (rng.standard_normal((C, C)) / 16).astype(np.float32),
        "phi_b": (rng.standard_normal((C,)) / 16).astype(np.float32),
        "W_w": (rng.standard_normal((C, C)) / 16).astype(np.float32),
        "W_b": (rng.standard_normal((C,)) / 16).astype(np.float32),
        "bn_gamma": np.ones((C,), np.float32),
        "bn_beta": np.zeros((C,), np.float32),
    }
    out = kernel(**fake)
    print("out", out.shape, out.dtype, float(np.abs(out).mean()))
